# revision 2
# baseline (speedup 1.0000x reference)
"""Trainium2 Bass kernel v2 for nn_MultiHeadAttention_90924457656943.

Strategy (8 NeuronCores, SPMD):
  - Head-shard: core j owns effective heads [8j, 8j+8) = columns
    [128j, 128j+128) of the second-projection weights.  The double
    projection is fused on-device per core: W_eff = W1 @ W2[:, cols]
    (computed transposed, N=512 fp32r, then PE-transposed into stationary
    chunks), so q^T/k^T/v^T for the core's heads over ALL 2048 rows come
    from a single projection of the replicated x^T.  No input collectives.
  - q/k W2 columns are host-permuted so heads land on 32-row strips:
    head h -> rows 32*(h%4) + 16*(h//4).  Group A (h<4) feeds the PE
    directly; group B is restaged to 32-aligned strips by SBUF DMA.
  - Scores: 4 heads per pass via PE row-tiling (tile_position (32t,0)),
    fp32r, kc chunks of 128 kpos into one [128, 2048] PSUM tile
    (bank-aligned 512-col slices).  Exp on ACT (scale=0.25, bias=-5.5
    folded; the e^-5.5 factor cancels in softmax) -> bf16.
  - attn@v: v^T is PE-transposed to natural layout with an interleaved
    ones column (denominators fall out of the same matmul), bf16; one
    matmul per head into a strip-packed PSUM accumulator
    (tile_position (0,32t)).
  - Per-batch-half: denominators -> reciprocal -> DRAM-broadcast mt
    tiles; 0/1 perm matmuls rebuild the module's quirky head-merge;
    normalize+cast bf16; one AllToAll per batch-half (overlapped with
    the other half's attention); out-projection in bf16.
"""

import os
import numpy as np
import ml_dtypes

DEBUG = os.environ.get("K2DEBUG", "0") == "1"

import concourse.bass as bass
import concourse.tile as tile
from concourse import bacc, mybir
from concourse.bass_utils import run_bass_kernel_spmd

F32 = mybir.dt.float32
F32R = mybir.dt.float32r
BF16 = mybir.dt.bfloat16
AF = mybir.ActivationFunctionType

B, S, F = 2, 1024, 1024
H = 16            # head dim
C = 64            # effective heads
NCORES = 8
HPC = 8           # heads per core
KC = 8            # 128-wide chunks of F
EXPM = 5.5        # exp shift; cancels in softmax, keeps bf16/etc in range

K1 = [0, 1, 4, 5, 8, 9, 12, 13]    # out 128-row blocks, src batch 0
K2 = [2, 3, 6, 7, 10, 11, 14, 15]  # src batch 1


def _round_tf32(x: np.ndarray) -> np.ndarray:
    u = np.ascontiguousarray(x, dtype=np.float32).view(np.uint32).copy()
    lsb = (u >> 12) & 1
    u += 0x7FF + lsb
    u &= np.uint32(0xFFFFF000)
    return u.view(np.float32)


def _head_perm() -> np.ndarray:
    """p[new_col] = old_col: head h, dim d -> 32*(h%4) + 16*(h//4) + d."""
    p = np.zeros(128, dtype=np.int64)
    for h in range(HPC):
        for d in range(H):
            p[32 * (h % 4) + 16 * (h // 4) + d] = 16 * h + d
    return p


def _perm_mats() -> np.ndarray:
    """P[(u,r)] [128,128]: rows 32m+hd -> cols 64u+16r+hd (any m)."""
    P = np.zeros((2, 4, 128, 128), dtype=np.float32)
    for u in range(2):
        for r in range(4):
            for m in range(4):
                for hd in range(H):
                    P[u, r, 32 * m + hd, 64 * u + 16 * r + hd] = 1.0
    return P.reshape(8, 128, 128)


def _build():
    nc = bacc.Bacc("TRN2", target_bir_lowering=False, debug=False,
                   num_devices=NCORES)

    xT = nc.dram_tensor("xT", [KC, 128, B * S], F32R, kind="ExternalInput")
    w1T = {p: nc.dram_tensor(f"w1T_{p}", [KC, 128, F], BF16,
                             kind="ExternalInput") for p in "qkv"}
    w2 = {p: nc.dram_tensor(f"w2_{p}", [128, KC, 128], BF16,
                            kind="ExternalInput") for p in "qkv"}
    beff = nc.dram_tensor("beff", [128, 4], F32, kind="ExternalInput")
    wo_d = nc.dram_tensor("wo_d", [KC, 128, F], BF16, kind="ExternalInput")
    wo_b = nc.dram_tensor("wo_b", [128, KC], F32, kind="ExternalInput")
    perm_d = nc.dram_tensor("perm_d", [128, 8, 128], BF16, kind="ExternalInput")
    ident_d = nc.dram_tensor("ident_d", [128, 128], F32R, kind="ExternalInput")
    outT = nc.dram_tensor("outT", [KC, 128, 256], F32, kind="ExternalOutput")
    dbg = {}
    if DEBUG:
        for nm in ("qT", "kT", "vT", "qTB", "kTB"):
            dbg[nm] = nc.dram_tensor(f"dbg_{nm}", [128, B * S], F32,
                                     kind="ExternalOutput")
        dbg["on"] = nc.dram_tensor("dbg_on", [2, 2, 128, S], BF16,
                                   kind="ExternalOutput")
        dbg["vn"] = nc.dram_tensor("dbg_vn", [16, 128, HPC * 17], BF16,
                                   kind="ExternalOutput")
        dbg["a2a"] = nc.dram_tensor("dbg_a2a", [2, NCORES, 128, 128], BF16,
                                    kind="ExternalOutput")

    a2a_in = [nc.dram_tensor(f"a2a{i}_in", [NCORES, 128, 128], BF16)
              for i in range(2)]
    a2a_out = [nc.dram_tensor(f"a2a{i}_out", [NCORES, 128, 128], BF16)
               for i in range(2)]
    RG = [list(range(NCORES))]

    def a2a(dst, src):
        nc.gpsimd.collective_compute(
            "AllToAll", mybir.AluOpType.bypass,
            ins=[src[:]], outs=[dst[:]], replica_groups=RG)

    from contextlib import ExitStack
    with tile.TileContext(nc) as tc, ExitStack() as stk:
        const = stk.enter_context(tc.tile_pool(name="const", bufs=1))
        ident = const.tile([128, 128], F32R, tag="ident")
        nc.sync.dma_start(out=ident[:], in_=ident_d.ap())
        beff_t = const.tile([128, 4], F32, tag="beff")
        nc.sync.dma_start(out=beff_t[:], in_=beff.ap())

        # ---------------- projections via W_eff ----------------
        qkv_pool = stk.enter_context(tc.tile_pool(name="qkv", bufs=1))
        qT = {}
        for nm in ("qT", "kT", "vT", "qTB", "kTB"):
            t_ = qkv_pool.tile([128, B * S], F32R, tag=nm)
            qT[nm] = t_
        vnat_pool = stk.enter_context(tc.tile_pool(name="vnat", bufs=1))
        vnat = []
        for ch in range(2 * KC):
            vt_ = vnat_pool.tile([128, HPC * 17], BF16, tag=f"vn{ch}")
            vnat.append(vt_)

        with tc.tile_pool(name="xp", bufs=1) as xpool, \
             tc.tile_pool(name="wef", bufs=1) as wefpool, \
             tc.tile_pool(name="w2p", bufs=2) as w2pool, \
             tc.tile_pool(name="w1p", bufs=6) as w1pool, \
             tc.tile_pool(name="wsb", bufs=2) as wsbpool, \
             tc.tile_pool(name="pps", bufs=1, space="PSUM") as ppsum, \
             tc.tile_pool(name="wps", bufs=2, space="PSUM") as wpsum, \
             tc.tile_pool(name="tps", bufs=2, space="PSUM") as tpsum:

            # DMA order = need order: v weights, q weights, x, k weights
            w2t = {}
            xt = []

            def load_w2(p):
                t = w2pool.tile([128, KC * 128], BF16, tag=f"w2{p}")
                nc.sync.dma_start(
                    out=t[:].rearrange("p (a f) -> p a f", a=KC),
                    in_=w2[p].ap())
                w2t[p] = t

            def load_x():
                for k in range(KC):
                    t = xpool.tile([128, B * S], F32R, tag=f"x{k}")
                    for q4 in range(4):
                        nc.gpsimd.dma_start(out=t[:, 512 * q4:512 * (q4 + 1)],
                                            in_=xT[k][:, 512 * q4:512 * (q4 + 1)])
                    xt.append(t)

            # phase 1: W_eff for all three projections (weights-paced)
            weff = {}
            for pi, p in enumerate("vqk"):
                load_w2(p)
                wefT = wsbpool.tile([128, F], F32R, tag=f"wefT{p}")
                for n2 in range(2):
                    ps = wpsum.tile([128, 512], F32, tag="wps")
                    for a in range(KC):
                        w1s = w1pool.tile([128, 512], BF16, tag="w1s")
                        nc.sync.dma_start(
                            out=w1s[:],
                            in_=w1T[p][a][:, 512 * n2:512 * (n2 + 1)])
                        nc.tensor.matmul(ps[:], w2t[p][:, 128 * a:128 * (a + 1)],
                                         w1s[:], start=(a == 0),
                                         stop=(a == KC - 1))
                    nc.scalar.copy(wefT[:, 512 * n2:512 * (n2 + 1)], ps[:])
                if p == "v":
                    load_x()    # on the gpsimd queue, parallel with weights
                wchunks = []
                for k in range(KC):
                    tp = tpsum.tile([128, 128], F32R, tag="tp")
                    nc.tensor.transpose(tp[:], wefT[:, 128 * k:128 * (k + 1)],
                                        ident[:])
                    wc = wefpool.tile([128, 128], F32R, tag=f"wef{p}{k}")
                    nc.vector.tensor_copy(wc[:], tp[:])
                    wchunks.append(wc)
                weff[p] = wchunks

            # phase 2: projections, k-outer (x-paced)
            for pi, p in enumerate("vqk"):
                dst = qT[{"q": "qT", "k": "kT", "v": "vT"}[p]]
                psl = []
                for m4 in range(4):
                    ps = ppsum.tile([128, 512], F32, tag=f"pj{m4}")
                    psl.append(ps)
                for k in range(KC):
                    for m4 in range(4):
                        nc.tensor.matmul(psl[m4][:], weff[p][k][:],
                                         xt[k][:, 512 * m4:512 * (m4 + 1)],
                                         start=(k == 0), stop=(k == KC - 1))
                for m4 in range(4):
                    nc.scalar.activation(dst[:, 512 * m4:512 * (m4 + 1)],
                                         psl[m4][:], AF.Identity,
                                         bias=beff_t[:, pi:pi + 1])
                if p == "v":
                    for ch in range(2 * KC):
                        tp = tpsum.tile([128, 128], F32R, tag="tp")
                        nc.tensor.transpose(
                            tp[:], dst[:, 128 * ch:128 * (ch + 1)], ident[:])
                        nc.vector.tensor_copy(
                            vnat[ch][:].rearrange("p (h d) -> p h d",
                                                  h=HPC)[:, :, 0:16],
                            tp[:].rearrange("p (h d) -> p h d", h=HPC))
                        nc.gpsimd.memset(vnat[ch][:, 16::17], 1.0)

            # restage group-B strips (head 4+t -> rows 32t)
            for src_nm, dst_nm in (("qT", "qTB"), ("kT", "kTB")):
                for t in range(4):
                    nc.sync.dma_start(
                        out=qT[dst_nm][32 * t:32 * t + 16, :],
                        in_=qT[src_nm][32 * t + 16:32 * t + 32, :])

        # ---------------- attention ----------------
        perm_sb = const.tile([128, 8 * 128], BF16, tag="perm")
        nc.gpsimd.dma_start(
            out=perm_sb[:].rearrange("p (n f) -> p n f", n=8), in_=perm_d.ap())
        wob_t = const.tile([128, KC], F32, tag="wob")
        nc.gpsimd.dma_start(out=wob_t[:], in_=wo_b.ap())
        wo_pool = stk.enter_context(tc.tile_pool(name="wo", bufs=1))
        wo_tiles = []
        for k in range(KC):
            wt = wo_pool.tile([128, F], BF16, tag=f"wo{k}")
            nc.gpsimd.dma_start(
                out=wt[:].rearrange("p (n f) -> p n f", n=KC), in_=wo_d[k])
            wo_tiles.append(wt)

        on_pool = stk.enter_context(tc.tile_pool(name="on", bufs=1))
        on_t = {}
        for b2 in range(2):
            for u in range(2):
                o = on_pool.tile([128, S], BF16, tag=f"on{b2}{u}")
                on_t[(b2, u)] = o
        rr_pool = stk.enter_context(tc.tile_pool(name="rrp", bufs=2))
        pperm = stk.enter_context(tc.tile_pool(name="pperm", bufs=2,
                                               space="PSUM"))
        otp = stk.enter_context(tc.tile_pool(name="otp", bufs=4))

        def perm_and_ship(b2):
            for e in range(2):
                for bb in range(2):
                    m = 2 * bb + e
                    pp = pperm.tile([128, 256], F32, tag="pp")
                    nmm = 0
                    for u in range(2):
                        for r in range(4):
                            pi2 = 4 * u + r
                            nc.tensor.matmul(
                                pp[:],
                                perm_sb[32 * m:32 * m + 16,
                                        128 * pi2:128 * pi2 + 128],
                                on_t[(b2, u)][32 * m:32 * m + 16, r::4],
                                start=(nmm == 0), stop=(nmm == 7),
                                tile_position=(32 * m, 0),
                                skip_group_check=True)
                            nmm += 1
                    ot = otp.tile([128, 256], BF16, tag="ot")
                    nc.vector.tensor_copy(ot[:], pp[:])
                    nc.sync.dma_start(out=a2a_in[b2][2 * m], in_=ot[:, 0:128])
                    nc.sync.dma_start(out=a2a_in[b2][2 * m + 1],
                                      in_=ot[:, 128:256])
            a2a(a2a_out[b2], a2a_in[b2])

        with tc.tile_pool(name="scp", bufs=2, space="PSUM") as scp, \
             tc.tile_pool(name="avp", bufs=2, space="PSUM") as avp, \
             tc.tile_pool(name="exp", bufs=4) as expool:

            iters = [(b2, u, q2, kc) for b2 in range(2) for u in range(2)
                     for q2 in range(2) for kc in range(KC)]
            st = {}
            prev = None
            for it in iters + [None]:
                if it is not None:
                    b2, u, q2, kc = it
                    qsrc = qT["qT"] if u == 0 else qT["qTB"]
                    ksrc = qT["kT"] if u == 0 else qT["kTB"]
                    sctiles = []
                    for hg in range(2):
                        sc = scp.tile([128, 1024], F32, tag="sc")
                        sctiles.append(sc)
                    for hg in range(2):
                        for t2 in range(2):
                            t = 2 * hg + t2
                            nc.tensor.matmul(
                                sctiles[hg][:, 512 * t2:512 * (t2 + 1)],
                                ksrc[32 * t:32 * t + 16,
                                     1024 * b2 + 128 * kc:
                                     1024 * b2 + 128 * (kc + 1)],
                                qsrc[32 * t:32 * t + 16,
                                     1024 * b2 + 512 * q2:
                                     1024 * b2 + 512 * (q2 + 1)],
                                start=True, stop=True,
                                tile_position=(32 * t, 0),
                                skip_group_check=True)
                    exs = []
                    for hg in range(2):
                        ex = expool.tile([128, 1024], BF16, tag="ex")
                        nc.scalar.activation(
                            ex[:], sctiles[hg][:],
                            AF.Exp, scale=0.25, bias=beff_t[:, 3:4])
                        exs.append(ex)
                    st[it] = exs
                if prev is not None:
                    pb2, pu, pq2, pkc = prev
                    pexs = st.pop(prev)
                    if pkc == 0:
                        av = avp.tile([128, 512], F32, tag="av")
                        st[(pb2, pu, pq2, "av")] = av
                    av = st[(pb2, pu, pq2, "av")]
                    for hg in range(2):
                        for t2 in range(2):
                            t = 2 * hg + t2
                            nc.tensor.matmul(
                                av[32 * t:32 * t + 17, :],
                                vnat[8 * pb2 + pkc][:, 17 * (4 * pu + t):
                                                    17 * (4 * pu + t) + 17],
                                pexs[hg][:, 512 * t2:512 * (t2 + 1)],
                                start=(pkc == 0), stop=(pkc == KC - 1),
                                tile_position=(0, 32 * t),
                                skip_group_check=True)
                    if pkc == KC - 1:
                        av = st.pop((pb2, pu, pq2, "av"))
                        rc = rr_pool.tile([128, 512], F32, tag="rc")
                        nc.vector.reciprocal_approx_fast(rc[:], av[:])
                        rr = rr_pool.tile([128, 512], F32, tag="rr")
                        nc.vector.stream_shuffle(rr[:], rc[:], [16] * 32)
                        for t in range(4):
                            nc.vector.tensor_mul(
                                on_t[(pb2, pu)][32 * t:32 * t + 16,
                                                512 * pq2:512 * (pq2 + 1)],
                                av[32 * t:32 * t + 16, :],
                                rr[32 * t:32 * t + 16, :])
                        if pq2 == 1 and pu == 1:
                            perm_and_ship(pb2)
                prev = it

        if DEBUG:
            for nm in ("qT", "kT", "vT", "qTB", "kTB"):
                nc.sync.dma_start(out=dbg[nm].ap(), in_=qT[nm][:].bitcast(F32))
            for b2 in range(2):
                for u in range(2):
                    nc.sync.dma_start(out=dbg["on"][b2, u], in_=on_t[(b2, u)][:])
                nc.sync.dma_start(out=dbg["a2a"][b2], in_=a2a_in[b2].ap())
            for ch in range(16):
                nc.sync.dma_start(out=dbg["vn"][ch], in_=vnat[ch][:])

        # ---------------- out-projection ----------------
        with tc.tile_pool(name="rop", bufs=1) as rop, \
             tc.tile_pool(name="fop", bufs=2) as fop, \
             tc.tile_pool(name="ops", bufs=2, space="PSUM") as ops:
            for half in range(2):
                ro = []
                for k in range(KC):
                    t = rop.tile([128, 128], BF16, tag=f"ro{half}{k}")
                    nc.sync.dma_start(out=t[:], in_=a2a_out[half][k])
                    ro.append(t)
                for n in range(KC):
                    ps = ops.tile([128, 128], F32, tag="ops")
                    for k in range(KC):
                        nc.tensor.matmul(ps[:],
                                         wo_tiles[k][:, 128 * n:128 * (n + 1)],
                                         ro[k][:], start=(k == 0),
                                         stop=(k == KC - 1))
                    fo = fop.tile([128, 128], F32, tag="fo")
                    nc.scalar.activation(fo[:], ps[:], AF.Identity,
                                         bias=wob_t[:, n:n + 1])
                    nc.sync.dma_start(out=outT[n][:, 128 * half:128 * (half + 1)],
                                      in_=fo[:])

    nc.finalize()
    return nc


_NC_CACHE = None


def _get_nc():
    global _NC_CACHE
    if _NC_CACHE is None:
        _NC_CACHE = _build()
    return _NC_CACHE


def kernel(x, wq_w, wq_b, wk_w, wk_b, wv_w, wv_b,
           vq_w, vq_b, vk_w, vk_b, vv_w, vv_b, wo_w, wo_b,
           _trace=False):
    nc = _get_nc()
    permc = _head_perm()

    xT_h = np.ascontiguousarray(
        _round_tf32(np.asarray(x, dtype=np.float32).reshape(B * S, F).T)
        .reshape(KC, 128, B * S))
    w1T_h = {p: np.ascontiguousarray(
        np.asarray(w).T.astype(ml_dtypes.bfloat16).reshape(KC, 128, F))
        for p, w in (("q", wq_w), ("k", wk_w), ("v", wv_w))}
    wo_h = np.ascontiguousarray(
        np.asarray(wo_w).astype(ml_dtypes.bfloat16).reshape(KC, 128, F))
    wo_b_h = np.ascontiguousarray(
        np.asarray(wo_b, dtype=np.float32).reshape(KC, 128).T)
    perm_h = np.ascontiguousarray(
        _perm_mats().transpose(1, 0, 2).astype(ml_dtypes.bfloat16))
    ident_h = np.eye(128, dtype=np.float32)

    w2_full = {"q": np.asarray(vq_w), "k": np.asarray(vk_w), "v": np.asarray(vv_w)}
    b1 = {"q": np.asarray(wq_b), "k": np.asarray(wk_b), "v": np.asarray(wv_b)}
    b2m = {"q": np.asarray(vq_b), "k": np.asarray(vk_b), "v": np.asarray(vv_b)}

    in_maps = []
    for j in range(NCORES):
        cols = np.arange(128 * j, 128 * (j + 1))
        m = {"xT": xT_h, "wo_d": wo_h, "wo_b": wo_b_h, "perm_d": perm_h,
             "ident_d": ident_h}
        m.update({f"w1T_{p}": w1T_h[p] for p in "qkv"})
        beff_cols = np.zeros((128, 4), dtype=np.float32)
        beff_cols[:, 3] = -EXPM
        for pi, p in enumerate("vqk"):
            w2c = w2_full[p][:, cols]
            be = b1[p].astype(np.float64) @ w2c + b2m[p][cols]
            if p != "v":
                w2c = w2c[:, permc]
                be = be[permc]
            m[f"w2_{p}"] = np.ascontiguousarray(
                w2c.astype(ml_dtypes.bfloat16).reshape(KC, 128, 128)
                .transpose(1, 0, 2))
            beff_cols[:, pi] = be.astype(np.float32)
        m["beff"] = beff_cols
        in_maps.append(m)

    kw = {}
    if _trace:
        import sys
        import types
        if "antenv.axon_hooks" not in sys.modules:
            import antenv
            mod = types.ModuleType("antenv.axon_hooks")
            mod._hook = None
            def _set(h):
                mod._hook = h
            def _get():
                return mod._hook
            mod.set_axon_ntff_profile_hook = _set
            mod.get_axon_ntff_profile_hook = _get
            sys.modules["antenv.axon_hooks"] = mod
            antenv.axon_hooks = mod
            from trn_agent_boot.trn_boot import _ntff_profile_via_ctypes
            _set(_ntff_profile_via_ctypes("/opt/axon/libaxon_pjrt.so"))
        kw = dict(trace=True, trace_cores=list(range(NCORES)))
    res = run_bass_kernel_spmd(nc, in_maps, core_ids=list(range(NCORES)), **kw)

    out = np.empty((B * S, F), dtype=np.float32)
    for i in range(NCORES):
        oT = res.results[i]["outT"]          # [8, 128, 256]
        full = oT.reshape(F, 256)
        out[128 * K1[i]:128 * K1[i] + 128] = full[:, 0:128].T
        out[128 * K2[i]:128 * K2[i] + 128] = full[:, 128:256].T
    if _trace:
        return out.reshape(B, S, F), res
    return out.reshape(B, S, F)


# revision 3
# speedup vs baseline: 1.0200x; 1.0200x over previous
"""Trainium2 Bass kernel v2 for nn_MultiHeadAttention_90924457656943.

Strategy (8 NeuronCores, SPMD):
  - Head-shard: core j owns effective heads [8j, 8j+8) = columns
    [128j, 128j+128) of the second-projection weights.  The double
    projection is fused on-device per core: W_eff = W1 @ W2[:, cols]
    (computed transposed, N=512 fp32r, then PE-transposed into stationary
    chunks), so q^T/k^T/v^T for the core's heads over ALL 2048 rows come
    from a single projection of the replicated x^T.  No input collectives.
  - q/k W2 columns are host-permuted so heads land on 32-row strips:
    head h -> rows 32*(h%4) + 16*(h//4).  Group A (h<4) feeds the PE
    directly; group B is restaged to 32-aligned strips by SBUF DMA.
  - Scores: 4 heads per pass via PE row-tiling (tile_position (32t,0)),
    fp32r, kc chunks of 128 kpos into one [128, 2048] PSUM tile
    (bank-aligned 512-col slices).  Exp on ACT (scale=0.25, bias=-5.5
    folded; the e^-5.5 factor cancels in softmax) -> bf16.
  - attn@v: v^T is PE-transposed to natural layout with an interleaved
    ones column (denominators fall out of the same matmul), bf16; one
    matmul per head into a strip-packed PSUM accumulator
    (tile_position (0,32t)).
  - Per-batch-half: denominators -> reciprocal -> DRAM-broadcast mt
    tiles; 0/1 perm matmuls rebuild the module's quirky head-merge;
    normalize+cast bf16; one AllToAll per batch-half (overlapped with
    the other half's attention); out-projection in bf16.
"""

import os
import numpy as np
import ml_dtypes

DEBUG = os.environ.get("K2DEBUG", "0") == "1"

import concourse.bass as bass
import concourse.tile as tile
from concourse import bacc, mybir
from concourse.bass_utils import run_bass_kernel_spmd

F32 = mybir.dt.float32
F32R = mybir.dt.float32r
BF16 = mybir.dt.bfloat16
AF = mybir.ActivationFunctionType

B, S, F = 2, 1024, 1024
H = 16            # head dim
C = 64            # effective heads
NCORES = 8
HPC = 8           # heads per core
KC = 8            # 128-wide chunks of F
EXPM = 5.5        # exp shift; cancels in softmax, keeps bf16/etc in range

K1 = [0, 1, 4, 5, 8, 9, 12, 13]    # out 128-row blocks, src batch 0
K2 = [2, 3, 6, 7, 10, 11, 14, 15]  # src batch 1


def _round_tf32(x: np.ndarray) -> np.ndarray:
    u = np.ascontiguousarray(x, dtype=np.float32).view(np.uint32).copy()
    lsb = (u >> 12) & 1
    u += 0x7FF + lsb
    u &= np.uint32(0xFFFFF000)
    return u.view(np.float32)


def _head_perm() -> np.ndarray:
    """p[new_col] = old_col: head h, dim d -> 32*(h%4) + 16*(h//4) + d."""
    p = np.zeros(128, dtype=np.int64)
    for h in range(HPC):
        for d in range(H):
            p[32 * (h % 4) + 16 * (h // 4) + d] = 16 * h + d
    return p


def _perm_mats() -> np.ndarray:
    """P[(u,r)] [128,128]: rows 32m+hd -> cols 64u+16r+hd (any m)."""
    P = np.zeros((2, 4, 128, 128), dtype=np.float32)
    for u in range(2):
        for r in range(4):
            for m in range(4):
                for hd in range(H):
                    P[u, r, 32 * m + hd, 64 * u + 16 * r + hd] = 1.0
    return P.reshape(8, 128, 128)


def _build():
    nc = bacc.Bacc("TRN2", target_bir_lowering=False, debug=False,
                   num_devices=NCORES)

    xT = nc.dram_tensor("xT", [KC, 128, B * S], F32R, kind="ExternalInput")
    w1T = {p: nc.dram_tensor(f"w1T_{p}", [KC, 128, F], BF16,
                             kind="ExternalInput") for p in "qkv"}
    w2 = {p: nc.dram_tensor(f"w2_{p}", [128, KC, 128], BF16,
                            kind="ExternalInput") for p in "qkv"}
    beff = nc.dram_tensor("beff", [128, 4], F32, kind="ExternalInput")
    wo_d = nc.dram_tensor("wo_d", [KC, 128, F], BF16, kind="ExternalInput")
    wo_b = nc.dram_tensor("wo_b", [128, KC], F32, kind="ExternalInput")
    perm_d = nc.dram_tensor("perm_d", [128, 8, 128], BF16, kind="ExternalInput")
    ident_d = nc.dram_tensor("ident_d", [128, 128], F32R, kind="ExternalInput")
    outT = nc.dram_tensor("outT", [KC, 128, 256], F32, kind="ExternalOutput")
    dbg = {}
    if DEBUG:
        for nm in ("qT", "kT", "vT", "qTB", "kTB"):
            dbg[nm] = nc.dram_tensor(f"dbg_{nm}", [128, B * S], F32,
                                     kind="ExternalOutput")
        dbg["on"] = nc.dram_tensor("dbg_on", [2, 2, 128, S], BF16,
                                   kind="ExternalOutput")
        dbg["vn"] = nc.dram_tensor("dbg_vn", [16, 128, HPC * 17], BF16,
                                   kind="ExternalOutput")
        dbg["a2a"] = nc.dram_tensor("dbg_a2a", [2, NCORES, 128, 128], BF16,
                                    kind="ExternalOutput")

    a2a_in = [nc.dram_tensor(f"a2a{i}_in", [NCORES, 128, 128], BF16)
              for i in range(2)]
    a2a_out = [nc.dram_tensor(f"a2a{i}_out", [NCORES, 128, 128], BF16)
               for i in range(2)]
    RG = [list(range(NCORES))]

    def a2a(dst, src):
        nc.gpsimd.collective_compute(
            "AllToAll", mybir.AluOpType.bypass,
            ins=[src[:]], outs=[dst[:]], replica_groups=RG)

    from contextlib import ExitStack
    with tile.TileContext(nc) as tc, ExitStack() as stk:
        const = stk.enter_context(tc.tile_pool(name="const", bufs=1))
        ident = const.tile([128, 128], F32R, tag="ident")
        nc.sync.dma_start(out=ident[:], in_=ident_d.ap())
        beff_t = const.tile([128, 4], F32, tag="beff")
        nc.sync.dma_start(out=beff_t[:], in_=beff.ap())

        # ---------------- projections via W_eff ----------------
        qkv_pool = stk.enter_context(tc.tile_pool(name="qkv", bufs=1))
        qT = {}
        for nm in ("qT", "kT", "vT", "qTB", "kTB"):
            t_ = qkv_pool.tile([128, B * S], F32R, tag=nm)
            qT[nm] = t_
        vnat_pool = stk.enter_context(tc.tile_pool(name="vnat", bufs=1))
        vnat = []
        for ch in range(2 * KC):
            vt_ = vnat_pool.tile([128, HPC * 17], BF16, tag=f"vn{ch}")
            vnat.append(vt_)

        with tc.tile_pool(name="xp", bufs=1) as xpool, \
             tc.tile_pool(name="wef", bufs=1) as wefpool, \
             tc.tile_pool(name="w2p", bufs=2) as w2pool, \
             tc.tile_pool(name="w1p", bufs=12) as w1pool, \
             tc.tile_pool(name="wsb", bufs=2) as wsbpool, \
             tc.tile_pool(name="pps", bufs=1, space="PSUM") as ppsum, \
             tc.tile_pool(name="wps", bufs=2, space="PSUM") as wpsum, \
             tc.tile_pool(name="tps", bufs=2, space="PSUM") as tpsum:

            # DMA order = need order: v weights, q weights, x, k weights
            w2t = {}
            xt = []

            def load_w2(p):
                t = w2pool.tile([128, KC * 128], BF16, tag=f"w2{p}")
                nc.sync.dma_start(
                    out=t[:].rearrange("p (a f) -> p a f", a=KC),
                    in_=w2[p].ap())
                w2t[p] = t

            def load_x():
                # first chunks via the idle gpsimd queue (issues immediately),
                # later chunks via sync (queue drains after the weight loads)
                for k in range(KC):
                    t = xpool.tile([128, B * S], F32R, tag=f"x{k}")
                    eng = nc.gpsimd if k < 5 else nc.sync
                    for q4 in range(4):
                        eng.dma_start(out=t[:, 512 * q4:512 * (q4 + 1)],
                                      in_=xT[k][:, 512 * q4:512 * (q4 + 1)])
                    xt.append(t)

            # phase 1: W_eff for all three projections (weights-paced)
            weff = {}
            for pi, p in enumerate("vqk"):
                load_w2(p)
                wefT = wsbpool.tile([128, F], F32R, tag=f"wefT{p}")
                for n2 in range(2):
                    ps = wpsum.tile([128, 512], F32, tag="wps")
                    for a in range(KC):
                        w1s = w1pool.tile([128, 512], BF16, tag="w1s")
                        nc.sync.dma_start(
                            out=w1s[:],
                            in_=w1T[p][a][:, 512 * n2:512 * (n2 + 1)])
                        nc.tensor.matmul(ps[:], w2t[p][:, 128 * a:128 * (a + 1)],
                                         w1s[:], start=(a == 0),
                                         stop=(a == KC - 1))
                    nc.scalar.copy(wefT[:, 512 * n2:512 * (n2 + 1)], ps[:])
                if p == "v":
                    load_x()    # on the gpsimd queue, parallel with weights
                wchunks = []
                for k in range(KC):
                    tp = tpsum.tile([128, 128], F32R, tag="tp")
                    nc.tensor.transpose(tp[:], wefT[:, 128 * k:128 * (k + 1)],
                                        ident[:])
                    wc = wefpool.tile([128, 128], F32R, tag=f"wef{p}{k}")
                    nc.vector.tensor_copy(wc[:], tp[:])
                    wchunks.append(wc)
                weff[p] = wchunks

            # phase 2: projections, k-outer (x-paced)
            for pi, p in enumerate("vqk"):
                dst = qT[{"q": "qT", "k": "kT", "v": "vT"}[p]]
                psl = []
                for m4 in range(4):
                    ps = ppsum.tile([128, 512], F32, tag=f"pj{m4}")
                    psl.append(ps)
                for k in range(KC):
                    for m4 in range(4):
                        nc.tensor.matmul(psl[m4][:], weff[p][k][:],
                                         xt[k][:, 512 * m4:512 * (m4 + 1)],
                                         start=(k == 0), stop=(k == KC - 1))
                for m4 in range(4):
                    nc.scalar.activation(dst[:, 512 * m4:512 * (m4 + 1)],
                                         psl[m4][:], AF.Identity,
                                         bias=beff_t[:, pi:pi + 1])
                if p == "v":
                    for ch in range(2 * KC):
                        tp = tpsum.tile([128, 128], F32R, tag="tp")
                        nc.tensor.transpose(
                            tp[:], dst[:, 128 * ch:128 * (ch + 1)], ident[:])
                        nc.vector.tensor_copy(
                            vnat[ch][:].rearrange("p (h d) -> p h d",
                                                  h=HPC)[:, :, 0:16],
                            tp[:].rearrange("p (h d) -> p h d", h=HPC))
                        nc.gpsimd.memset(vnat[ch][:, 16::17], 1.0)

            # restage group-B strips (head 4+t -> rows 32t)
            for src_nm, dst_nm in (("qT", "qTB"), ("kT", "kTB")):
                for t in range(4):
                    nc.sync.dma_start(
                        out=qT[dst_nm][32 * t:32 * t + 16, :],
                        in_=qT[src_nm][32 * t + 16:32 * t + 32, :])

        # ---------------- attention ----------------
        perm_sb = const.tile([128, 8 * 128], BF16, tag="perm")
        nc.gpsimd.dma_start(
            out=perm_sb[:].rearrange("p (n f) -> p n f", n=8), in_=perm_d.ap())
        wob_t = const.tile([128, KC], F32, tag="wob")
        nc.gpsimd.dma_start(out=wob_t[:], in_=wo_b.ap())
        wo_pool = stk.enter_context(tc.tile_pool(name="wo", bufs=1))
        wo_tiles = []
        for k in range(KC):
            wt = wo_pool.tile([128, F], BF16, tag=f"wo{k}")
            nc.gpsimd.dma_start(
                out=wt[:].rearrange("p (n f) -> p n f", n=KC), in_=wo_d[k])
            wo_tiles.append(wt)

        on_pool = stk.enter_context(tc.tile_pool(name="on", bufs=1))
        on_t = {}
        for b2 in range(2):
            for u in range(2):
                o = on_pool.tile([128, S], BF16, tag=f"on{b2}{u}")
                on_t[(b2, u)] = o
        rr_pool = stk.enter_context(tc.tile_pool(name="rrp", bufs=2))
        pperm = stk.enter_context(tc.tile_pool(name="pperm", bufs=2,
                                               space="PSUM"))
        otp = stk.enter_context(tc.tile_pool(name="otp", bufs=4))

        def perm_and_ship(b2):
            for e in range(2):
                for bb in range(2):
                    m = 2 * bb + e
                    pp = pperm.tile([128, 256], F32, tag="pp")
                    nmm = 0
                    for u in range(2):
                        for r in range(4):
                            pi2 = 4 * u + r
                            nc.tensor.matmul(
                                pp[:],
                                perm_sb[32 * m:32 * m + 16,
                                        128 * pi2:128 * pi2 + 128],
                                on_t[(b2, u)][32 * m:32 * m + 16, r::4],
                                start=(nmm == 0), stop=(nmm == 7),
                                tile_position=(32 * m, 0),
                                skip_group_check=True)
                            nmm += 1
                    ot = otp.tile([128, 256], BF16, tag="ot")
                    nc.vector.tensor_copy(ot[:], pp[:])
                    nc.sync.dma_start(out=a2a_in[b2][2 * m], in_=ot[:, 0:128])
                    nc.sync.dma_start(out=a2a_in[b2][2 * m + 1],
                                      in_=ot[:, 128:256])
            a2a(a2a_out[b2], a2a_in[b2])

        with tc.tile_pool(name="scp", bufs=2, space="PSUM") as scp, \
             tc.tile_pool(name="avp", bufs=2, space="PSUM") as avp, \
             tc.tile_pool(name="exp", bufs=4) as expool:

            iters = [(b2, u, q2, kc) for b2 in range(2) for u in range(2)
                     for q2 in range(2) for kc in range(KC)]
            st = {}
            prev = None
            for it in iters + [None]:
                if it is not None:
                    b2, u, q2, kc = it
                    qsrc = qT["qT"] if u == 0 else qT["qTB"]
                    ksrc = qT["kT"] if u == 0 else qT["kTB"]
                    sctiles = []
                    for hg in range(2):
                        sc = scp.tile([128, 1024], F32, tag="sc")
                        sctiles.append(sc)
                    for hg in range(2):
                        for t2 in range(2):
                            t = 2 * hg + t2
                            nc.tensor.matmul(
                                sctiles[hg][:, 512 * t2:512 * (t2 + 1)],
                                ksrc[32 * t:32 * t + 16,
                                     1024 * b2 + 128 * kc:
                                     1024 * b2 + 128 * (kc + 1)],
                                qsrc[32 * t:32 * t + 16,
                                     1024 * b2 + 512 * q2:
                                     1024 * b2 + 512 * (q2 + 1)],
                                start=True, stop=True,
                                tile_position=(32 * t, 0),
                                skip_group_check=True)
                    exs = []
                    for hg in range(2):
                        ex = expool.tile([128, 1024], BF16, tag="ex")
                        nc.scalar.activation(
                            ex[:], sctiles[hg][:],
                            AF.Exp, scale=0.25, bias=beff_t[:, 3:4])
                        exs.append(ex)
                    st[it] = exs
                if prev is not None:
                    pb2, pu, pq2, pkc = prev
                    pexs = st.pop(prev)
                    if pkc == 0:
                        av = avp.tile([128, 512], F32, tag="av")
                        st[(pb2, pu, pq2, "av")] = av
                    av = st[(pb2, pu, pq2, "av")]
                    for hg in range(2):
                        for t2 in range(2):
                            t = 2 * hg + t2
                            nc.tensor.matmul(
                                av[32 * t:32 * t + 17, :],
                                vnat[8 * pb2 + pkc][:, 17 * (4 * pu + t):
                                                    17 * (4 * pu + t) + 17],
                                pexs[hg][:, 512 * t2:512 * (t2 + 1)],
                                start=(pkc == 0), stop=(pkc == KC - 1),
                                tile_position=(0, 32 * t),
                                skip_group_check=True)
                    if pkc == KC - 1:
                        av = st.pop((pb2, pu, pq2, "av"))
                        rc = rr_pool.tile([128, 512], F32, tag="rc")
                        nc.vector.reciprocal_approx_fast(rc[:], av[:])
                        rr = rr_pool.tile([128, 512], F32, tag="rr")
                        nc.vector.stream_shuffle(rr[:], rc[:], [16] * 32)
                        for t in range(4):
                            nc.vector.tensor_mul(
                                on_t[(pb2, pu)][32 * t:32 * t + 16,
                                                512 * pq2:512 * (pq2 + 1)],
                                av[32 * t:32 * t + 16, :],
                                rr[32 * t:32 * t + 16, :])
                        if pq2 == 1 and pu == 1:
                            perm_and_ship(pb2)
                prev = it

        if DEBUG:
            for nm in ("qT", "kT", "vT", "qTB", "kTB"):
                nc.sync.dma_start(out=dbg[nm].ap(), in_=qT[nm][:].bitcast(F32))
            for b2 in range(2):
                for u in range(2):
                    nc.sync.dma_start(out=dbg["on"][b2, u], in_=on_t[(b2, u)][:])
                nc.sync.dma_start(out=dbg["a2a"][b2], in_=a2a_in[b2].ap())
            for ch in range(16):
                nc.sync.dma_start(out=dbg["vn"][ch], in_=vnat[ch][:])

        # ---------------- out-projection ----------------
        with tc.tile_pool(name="rop", bufs=1) as rop, \
             tc.tile_pool(name="fop", bufs=2) as fop, \
             tc.tile_pool(name="ops", bufs=2, space="PSUM") as ops:
            for half in range(2):
                ro = []
                for k in range(KC):
                    t = rop.tile([128, 128], BF16, tag=f"ro{half}{k}")
                    nc.gpsimd.dma_start(out=t[:], in_=a2a_out[half][k])
                    ro.append(t)
                for n in range(KC):
                    ps = ops.tile([128, 128], F32, tag="ops")
                    for k in range(KC):
                        nc.tensor.matmul(ps[:],
                                         wo_tiles[k][:, 128 * n:128 * (n + 1)],
                                         ro[k][:], start=(k == 0),
                                         stop=(k == KC - 1))
                    fo = fop.tile([128, 128], F32, tag="fo")
                    nc.scalar.activation(fo[:], ps[:], AF.Identity,
                                         bias=wob_t[:, n:n + 1])
                    nc.sync.dma_start(out=outT[n][:, 128 * half:128 * (half + 1)],
                                      in_=fo[:])

    nc.finalize()
    return nc


_NC_CACHE = None


def _get_nc():
    global _NC_CACHE
    if _NC_CACHE is None:
        _NC_CACHE = _build()
    return _NC_CACHE


def kernel(x, wq_w, wq_b, wk_w, wk_b, wv_w, wv_b,
           vq_w, vq_b, vk_w, vk_b, vv_w, vv_b, wo_w, wo_b,
           _trace=False):
    nc = _get_nc()
    permc = _head_perm()

    xT_h = np.ascontiguousarray(
        _round_tf32(np.asarray(x, dtype=np.float32).reshape(B * S, F).T)
        .reshape(KC, 128, B * S))
    w1T_h = {p: np.ascontiguousarray(
        np.asarray(w).T.astype(ml_dtypes.bfloat16).reshape(KC, 128, F))
        for p, w in (("q", wq_w), ("k", wk_w), ("v", wv_w))}
    wo_h = np.ascontiguousarray(
        np.asarray(wo_w).astype(ml_dtypes.bfloat16).reshape(KC, 128, F))
    wo_b_h = np.ascontiguousarray(
        np.asarray(wo_b, dtype=np.float32).reshape(KC, 128).T)
    perm_h = np.ascontiguousarray(
        _perm_mats().transpose(1, 0, 2).astype(ml_dtypes.bfloat16))
    ident_h = np.eye(128, dtype=np.float32)

    w2_full = {"q": np.asarray(vq_w), "k": np.asarray(vk_w), "v": np.asarray(vv_w)}
    b1 = {"q": np.asarray(wq_b), "k": np.asarray(wk_b), "v": np.asarray(wv_b)}
    b2m = {"q": np.asarray(vq_b), "k": np.asarray(vk_b), "v": np.asarray(vv_b)}

    in_maps = []
    for j in range(NCORES):
        cols = np.arange(128 * j, 128 * (j + 1))
        m = {"xT": xT_h, "wo_d": wo_h, "wo_b": wo_b_h, "perm_d": perm_h,
             "ident_d": ident_h}
        m.update({f"w1T_{p}": w1T_h[p] for p in "qkv"})
        beff_cols = np.zeros((128, 4), dtype=np.float32)
        beff_cols[:, 3] = -EXPM
        for pi, p in enumerate("vqk"):
            w2c = w2_full[p][:, cols]
            be = b1[p].astype(np.float64) @ w2c + b2m[p][cols]
            if p != "v":
                w2c = w2c[:, permc]
                be = be[permc]
            m[f"w2_{p}"] = np.ascontiguousarray(
                w2c.astype(ml_dtypes.bfloat16).reshape(KC, 128, 128)
                .transpose(1, 0, 2))
            beff_cols[:, pi] = be.astype(np.float32)
        m["beff"] = beff_cols
        in_maps.append(m)

    kw = {}
    if _trace:
        import sys
        import types
        if "antenv.axon_hooks" not in sys.modules:
            import antenv
            mod = types.ModuleType("antenv.axon_hooks")
            mod._hook = None
            def _set(h):
                mod._hook = h
            def _get():
                return mod._hook
            mod.set_axon_ntff_profile_hook = _set
            mod.get_axon_ntff_profile_hook = _get
            sys.modules["antenv.axon_hooks"] = mod
            antenv.axon_hooks = mod
            from trn_agent_boot.trn_boot import _ntff_profile_via_ctypes
            _set(_ntff_profile_via_ctypes("/opt/axon/libaxon_pjrt.so"))
        kw = dict(trace=True, trace_cores=list(range(NCORES)))
    res = run_bass_kernel_spmd(nc, in_maps, core_ids=list(range(NCORES)), **kw)

    out = np.empty((B * S, F), dtype=np.float32)
    for i in range(NCORES):
        oT = res.results[i]["outT"]          # [8, 128, 256]
        full = oT.reshape(F, 256)
        out[128 * K1[i]:128 * K1[i] + 128] = full[:, 0:128].T
        out[128 * K2[i]:128 * K2[i] + 128] = full[:, 128:256].T
    if _trace:
        return out.reshape(B, S, F), res
    return out.reshape(B, S, F)
